# revision 1
# baseline (speedup 1.0000x reference)
"""Trainium2 Bass kernel for the 6-level hierarchical Choquet integral tree.

Tree: 16-ary, depth 6, 16.7M leaves. Each node computes a 2-additive Choquet
integral of its 16 children: softmax(theta) over 136 coeffs (16 singles +
120 pair-mins), dot with [children ; pairwise mins].

Sharding: 8 cores, each owns a contiguous subtree of 2M leaves and computes
levels 1-4 on device (output: 32 level-4 values per core). Host finishes the
tiny levels 5-6 (17 nodes) in numpy.

Device layout ("layout A"): nodes on partitions, G nodes per partition row,
features along the free dim. Pair mins are computed with 15 shifted
tensor_tensor(min) ops over the 16 children (pairs grouped by distance d);
theta columns are pre-permuted on the host into the matching d-major order
(softmax and the weighted sum are permutation invariant, so this is exact).
"""

import os

import numpy as np

import concourse.bass as bass
import concourse.mybir as mybir
import concourse.tile as tile
from concourse import bacc
from concourse.bass_utils import run_bass_kernel_spmd

B = 16
NPAIR = 120
NF = B + NPAIR  # 136
NCORE = 8
LEAF_PER_CORE = 16**6 // NCORE  # 2,097,152
# per-core node counts for on-device levels 1..4
LN = [LEAF_PER_CORE // (B**l) for l in range(1, 5)]  # [131072, 8192, 512, 32]

# (P partitions, G nodes per partition row) per level
LEVEL_PG = [(128, 16), (128, 16), (128, 4), (32, 1)]

_F32 = mybir.dt.float32


def _dmajor_perm() -> np.ndarray:
    """Column permutation mapping natural theta order -> [singles, pairs in
    d-major order], where pair (i, i+d) sits at offset off_d + i."""
    II, JJ = np.triu_indices(B, k=1)
    nat = {(int(i), int(j)): p for p, (i, j) in enumerate(zip(II, JJ))}
    perm = list(range(B))
    for d in range(1, B):
        for i in range(B - d):
            perm.append(B + nat[(i, i + d)])
    assert len(perm) == NF
    return np.array(perm, dtype=np.int64)


PERM = _dmajor_perm()
# off_d: start of distance-d block within the 120 pair columns
OFFD = np.concatenate([[0], np.cumsum([B - d for d in range(1, B)])])


def _build_program() -> bass.Bass:
    nc = bacc.Bacc("TRN2", target_bir_lowering=False, debug=False)

    x_d = nc.dram_tensor("x", [LEAF_PER_CORE], _F32, kind="ExternalInput")
    th_d = [
        nc.dram_tensor(f"t{l + 1}", [LN[l] * NF], _F32, kind="ExternalInput")
        for l in range(4)
    ]
    v_d = [nc.dram_tensor(f"v{l + 1}", [LN[l]], _F32) for l in range(3)]
    o4 = nc.dram_tensor("o4", [LN[3]], _F32, kind="ExternalOutput")

    srcs = [x_d.ap()] + [v.ap() for v in v_d]
    dsts = [v.ap() for v in v_d] + [o4.ap()]
    th_aps = [t.ap() for t in th_d]

    with tile.TileContext(nc) as tc:
        _kernel_body(nc, tc, th_aps, srcs, dsts, LN, LEVEL_PG)
    nc.compile()
    return nc


def _kernel_body(nc, tc, th_aps, srcs, dsts, lns, level_pg) -> None:
    if True:
        with (
            tc.tile_pool(name="th", bufs=3) as thp,
            tc.tile_pool(name="xs", bufs=3) as xsp,
            tc.tile_pool(name="e", bufs=2) as ep,
            tc.tile_pool(name="buf", bufs=2) as bp,
            tc.tile_pool(name="small", bufs=3) as sp,
        ):
            for lvl in range(len(lns)):
                P, G = level_pg[lvl]
                n_nodes = lns[lvl]
                ntile = P * G
                T = n_nodes // ntile
                th_src = th_aps[lvl].rearrange(
                    "(t p f) -> t p f", p=P, f=G * NF
                )
                xs_src = srcs[lvl].rearrange("(t p q) -> t p q", p=P, q=G * B)
                out_dst = dsts[lvl].rearrange("(t p g) -> t p g", p=P, g=G)

                for t in range(T):
                    th_t = thp.tile([P, G * NF], _F32, tag="th")
                    nc.gpsimd.dma_start(out=th_t[:], in_=th_src[t])
                    xs_t = xsp.tile([P, G * B], _F32, tag="xs")
                    nc.gpsimd.dma_start(out=xs_t[:], in_=xs_src[t])

                    e_t = ep.tile([P, G * NF], _F32, tag="e")
                    nc.scalar.activation(
                        e_t[:], th_t[:], mybir.ActivationFunctionType.Exp
                    )

                    buf_t = bp.tile([P, G * NF], _F32, tag="buf")
                    b3 = buf_t[:].rearrange("p (g f) -> p g f", g=G)
                    x3 = xs_t[:].rearrange("p (g f) -> p g f", g=G)
                    # pair mins, distance-major
                    for d in range(1, B):
                        c = B - d
                        o = B + int(OFFD[d - 1])
                        nc.vector.tensor_tensor(
                            b3[:, :, o : o + c],
                            x3[:, :, 0:c],
                            x3[:, :, d:B],
                            op=mybir.AluOpType.min,
                        )

                    e3 = e_t[:].rearrange("p (g f) -> p g f", g=G)
                    den_t = sp.tile([P, G], _F32, tag="den")
                    nc.vector.tensor_reduce(
                        den_t[:], e3, axis=mybir.AxisListType.X, op=mybir.AluOpType.add
                    )
                    # prod: singles e*xs land in buf[:,:, :16]; pairs in place
                    nc.vector.tensor_tensor(
                        b3[:, :, 0:B], e3[:, :, 0:B], x3[:, :, :],
                        op=mybir.AluOpType.mult,
                    )
                    nc.vector.tensor_tensor(
                        b3[:, :, B:], b3[:, :, B:], e3[:, :, B:],
                        op=mybir.AluOpType.mult,
                    )
                    dot_t = sp.tile([P, G], _F32, tag="dot")
                    nc.vector.tensor_reduce(
                        dot_t[:], b3, axis=mybir.AxisListType.X, op=mybir.AluOpType.add
                    )
                    rcp_t = sp.tile([P, G], _F32, tag="rcp")
                    nc.vector.reciprocal(rcp_t[:], den_t[:])
                    out_t = sp.tile([P, G], _F32, tag="out")
                    nc.vector.tensor_mul(out_t[:], dot_t[:], rcp_t[:])
                    nc.gpsimd.dma_start(out=out_dst[t], in_=out_t[:])


_PROG_CACHE: bass.Bass | None = None
LAST_RESULTS = None  # BassKernelResults of the most recent kernel() call


def _ensure_ntff_hook() -> None:
    """Provide antenv.axon_hooks + the ctypes NTFF hook when the image
    lacks them, so trace=True produces a perfetto profile under axon."""
    import contextlib
    import ctypes
    import sys
    import types

    try:
        from antenv.axon_hooks import get_axon_ntff_profile_hook  # noqa: F401

        return
    except ImportError:
        pass

    import antenv
    import concourse.bass_utils as bu

    holder = {"h": None}
    mod = types.ModuleType("antenv.axon_hooks")
    mod.set_axon_ntff_profile_hook = lambda h: holder.__setitem__("h", h)
    mod.get_axon_ntff_profile_hook = lambda: holder["h"]
    sys.modules["antenv.axon_hooks"] = mod
    antenv.axon_hooks = mod
    bu.upload_artifacts = lambda tmpdir: ""  # no artifact bucket here

    so_path = "/opt/axon/libaxon_pjrt.so"
    try:
        lib = ctypes.CDLL(so_path)
    except OSError:
        return
    if not hasattr(lib, "axon_start_nrt_profile"):
        return
    lib.axon_start_nrt_profile.argtypes = [
        ctypes.POINTER(ctypes.c_int64),
        ctypes.c_size_t,
    ]
    lib.axon_start_nrt_profile.restype = ctypes.c_int64
    lib.axon_stop_nrt_profile.argtypes = [ctypes.c_char_p]
    lib.axon_stop_nrt_profile.restype = ctypes.c_int64

    @contextlib.contextmanager
    def _hook(output_dir, device_ids):
        import jax

        jax.devices()
        if device_ids:
            ids = (ctypes.c_int64 * len(device_ids))(*device_ids)
            rc = lib.axon_start_nrt_profile(ids, len(device_ids))
        else:
            rc = lib.axon_start_nrt_profile(None, 0)
        if rc != 0:
            raise RuntimeError(f"axon_start_nrt_profile rc={rc}")
        try:
            yield
        finally:
            n = lib.axon_stop_nrt_profile(str(output_dir).encode())
            print(f"profile: {n} file(s) written to {output_dir}")

    mod.set_axon_ntff_profile_hook(_hook)


def _choquet_np(vals: np.ndarray, theta: np.ndarray) -> np.ndarray:
    II, JJ = np.triu_indices(B, k=1)
    n = theta.shape[0]
    xs = vals.reshape(n, B).astype(np.float64)
    t = theta.astype(np.float64)
    e = np.exp(t - t.max(axis=1, keepdims=True))
    m = e / e.sum(axis=1, keepdims=True)
    mins = np.minimum(xs[:, II], xs[:, JJ])
    return (m[:, :B] * xs).sum(axis=1) + (m[:, B:] * mins).sum(axis=1)


def kernel(x, theta1, theta2, theta3, theta4, theta5, theta6) -> np.ndarray:
    global _PROG_CACHE, LAST_RESULTS
    x = np.ascontiguousarray(np.asarray(x, dtype=np.float32).reshape(-1))
    ths = []
    for th in (theta1, theta2, theta3, theta4):
        th = np.asarray(th, dtype=np.float32)
        ths.append(np.ascontiguousarray(th[:, PERM]))

    if _PROG_CACHE is None:
        _PROG_CACHE = _build_program()
    nc = _PROG_CACHE

    in_maps = []
    for c in range(NCORE):
        m = {"x": x[c * LEAF_PER_CORE : (c + 1) * LEAF_PER_CORE]}
        for l in range(4):
            rows = LN[l]
            m[f"t{l + 1}"] = ths[l][c * rows : (c + 1) * rows].reshape(-1)
        in_maps.append(m)

    trace = os.environ.get("BASS_KERNEL_TRACE", "0") == "1"
    if trace:
        _ensure_ntff_hook()
    res = run_bass_kernel_spmd(nc, in_maps, list(range(NCORE)), trace=trace)
    LAST_RESULTS = res

    l4 = np.concatenate([res.results[c]["o4"].reshape(-1) for c in range(NCORE)])
    l5 = _choquet_np(l4, np.asarray(theta5, dtype=np.float32))
    l6 = _choquet_np(l5, np.asarray(theta6, dtype=np.float32))
    return l6.astype(np.float32).reshape((1,))



# revision 3
# speedup vs baseline: 2.2648x; 2.2648x over previous
"""Trainium2 Bass kernel for the 6-level hierarchical Choquet integral tree.

Tree: 16-ary, depth 6, 16.7M leaves. Each node: softmax(theta) over 136
coeffs (16 singles + 120 pair-mins), dot with [children ; pairwise mins].

v2 design:
- Host precomputes m = softmax(theta) in f32 (theta-only -> static), ships
  bf16, feature-major per tile row: col = f*16 + g (g = node-in-row).
- Pair mins via 8 wrapped-rotation ops: d=1..7 pairs (i, (i+d)%16) i=0..15,
  d=8 i=0..7 -> covers all 120 unordered pairs once. xs is shipped
  duplicated ([xs|xs], 512 cols) so every rotation is a contiguous 2x-mode
  tensor_tensor(min).
- Dot via 2 contiguous mults + binary tree over feature rows
  (136->68->34->17) + one strided grouped reduce -> [p, 16] f32.
- Levels 1-2 on device (8 cores x 2M leaves); levels 3-6 on host (4369
  nodes, numpy).
- Level-1 -> level-2 handoff stays in SBUF; the f32->bf16 cast copy on the
  Act engine writes strided so the next level's feature-major layout (and
  its wrap duplicate) appear for free.
"""

import os

import numpy as np

import concourse.bass as bass
import concourse.mybir as mybir
import concourse.tile as tile
from concourse import bacc
from concourse.bass_utils import run_bass_kernel_spmd

B = 16
NPAIR = 120
NF = B + NPAIR  # 136
NCORE = 8
LEAF_PER_CORE = 16**6 // NCORE  # 2,097,152
N1 = LEAF_PER_CORE // B         # 131072 level-1 nodes/core
N2 = N1 // B                    # 8192  level-2 nodes/core
T1 = 64                         # level-1 tiles of 2048 nodes (128p x 16)
Q2 = 4                          # level-2 sub-tiles of 2048 nodes

_F32 = mybir.dt.float32
_BF = mybir.dt.bfloat16


def _pair_perm() -> np.ndarray:
    """Map wrapped-rotation pair position q -> natural pair index (0..119).

    Position q = (d-1)*16 + i for d=1..7 (i=0..15), then 112+i for d=8
    (i=0..7); pair is (i, (i+d) % 16)."""
    II, JJ = np.triu_indices(B, k=1)
    nat = {(int(a), int(b)): p for p, (a, b) in enumerate(zip(II, JJ))}
    perm = []
    for d in range(1, 9):
        for i in range(B if d < 8 else 8):
            j = (i + d) % B
            a, b = min(i, j), max(i, j)
            perm.append(nat[(a, b)])
    assert len(perm) == NPAIR and len(set(perm)) == NPAIR
    return np.array(perm, dtype=np.int64)


PAIR_PERM = _pair_perm()


def _kernel_tile(nc, pools, m_src, xs_dst_dup, xs2_ap, out_cb):
    """One 2048-node Choquet tile.

    m_src: DRAM AP [128, 2176] bf16 (feature-major softmax weights).
    xs_dst_dup: None, or (dram_ap) to DMA [128, 512] duplicated children.
    xs2_ap: SBUF AP [128, 512] holding [xs|xs] f-major (if xs_dst_dup is
            None the caller already filled it).
    out_cb(dot_ap): consume the [128, 16] f32 result."""
    mp, minp, pp, tp, sp = pools
    W = B * NF  # 2176

    m_t = mp.tile([128, W], _BF, tag="m")
    nc.sync.dma_start(out=m_t[:], in_=m_src)
    if xs_dst_dup is not None:
        nc.sync.dma_start(out=xs2_ap, in_=xs_dst_dup)

    # pair mins: 8 wrapped rotations, all contiguous
    mn_t = minp.tile([128, NPAIR * B], _BF, tag="mn")
    for d in range(1, 9):
        c = B if d < 8 else 8
        o = (d - 1) * B * B
        nc.vector.tensor_tensor(
            mn_t[:, o : o + c * B],
            xs2_ap[:, 0 : c * B],
            xs2_ap[:, d * B : d * B + c * B],
            op=mybir.AluOpType.min,
        )

    # products: P[0:256] = m_s * xs, P[256:2176] = m_p * mins
    p_t = pp.tile([128, W], _BF, tag="p")
    nc.vector.tensor_tensor(
        p_t[:, 0 : B * B], m_t[:, 0 : B * B], xs2_ap[:, 0 : B * B],
        op=mybir.AluOpType.mult,
    )
    nc.vector.tensor_tensor(
        p_t[:, B * B :], m_t[:, B * B :], mn_t[:],
        op=mybir.AluOpType.mult,
    )

    # tree-reduce 136 feature rows -> 68 -> 34 -> 17, then grouped reduce
    t1 = tp.tile([128, 68 * B], _BF, tag="t1")
    nc.vector.tensor_tensor(
        t1[:], p_t[:, 0 : 68 * B], p_t[:, 68 * B :], op=mybir.AluOpType.add)
    t2 = tp.tile([128, 34 * B], _BF, tag="t2")
    nc.vector.tensor_tensor(
        t2[:], t1[:, 0 : 34 * B], t1[:, 34 * B :], op=mybir.AluOpType.add)
    t3 = tp.tile([128, 17 * B], _BF, tag="t3")
    nc.vector.tensor_tensor(
        t3[:], t2[:, 0 : 17 * B], t2[:, 17 * B :], op=mybir.AluOpType.add)
    dot = sp.tile([128, B], _F32, tag="dot")
    nc.vector.tensor_reduce(
        dot[:],
        t3[:].rearrange("p (r g) -> p g r", r=17),
        axis=mybir.AxisListType.X,
        op=mybir.AluOpType.add,
    )
    out_cb(dot)


def _build_program() -> bass.Bass:
    nc = bacc.Bacc("TRN2", target_bir_lowering=False, debug=False)

    m1_d = nc.dram_tensor("m1", [T1 * 128 * B * NF], _BF, kind="ExternalInput")
    x_d = nc.dram_tensor("xd", [T1 * 128 * 2 * 256], _BF, kind="ExternalInput")
    m2_d = nc.dram_tensor("m2", [Q2 * 128 * B * NF], _BF, kind="ExternalInput")
    o2_d = nc.dram_tensor("o2", [128 * 64], _F32, kind="ExternalOutput")

    m1_src = m1_d.ap().rearrange("(t p f) -> t p f", p=128, f=B * NF)
    x_src = x_d.ap().rearrange("(t p f) -> t p f", p=128, f=512)
    m2_src = m2_d.ap().rearrange("(q p f) -> q p f", p=128, f=B * NF)
    o2_dst = o2_d.ap().rearrange("(p t) -> p t", t=64)

    with tile.TileContext(nc) as tc:
        with (
            tc.tile_pool(name="m", bufs=3) as mp,
            tc.tile_pool(name="xs", bufs=3) as xsp,
            tc.tile_pool(name="mn", bufs=2) as minp,
            tc.tile_pool(name="pr", bufs=2) as pp,
            tc.tile_pool(name="tr", bufs=2) as tp,
            tc.tile_pool(name="sm", bufs=4) as sp,
            tc.tile_pool(name="v1", bufs=1) as v1p,
        ):
            pools = (mp, minp, pp, tp, sp)
            # level-1 -> level-2 staging: per sub-tile q: [xs(256)|xs(256)]
            v1buf = v1p.tile([128, Q2 * 512], _BF, tag="v1buf")

            def mk_store(t):
                q, tl = t // 16, t % 16

                def store(dot):
                    # f32 [p, 16] -> bf16 strided (f-major for level 2),
                    # written twice (wrap duplicate), on the Act engine
                    blk = v1buf[:, q * 512 : (q + 1) * 512].rearrange(
                        "p (h i g) -> p h i g", h=2, i=B)
                    for h in (0, 1):
                        nc.scalar.activation(
                            blk[:, h, :, tl : tl + 1],
                            dot[:].rearrange("p (i o) -> p i o", o=1),
                            mybir.ActivationFunctionType.Copy,
                        )
                return store

            xs2_l1 = [None] * T1
            for t in range(T1):
                xs2_t = xsp.tile([128, 512], _BF, tag="xs2")
                xs2_l1[t] = xs2_t
                _kernel_tile(nc, pools, m1_src[t], x_src[t], xs2_t[:],
                             mk_store(t))

            for q in range(Q2):
                def store2(dot, q=q):
                    nc.sync.dma_start(
                        out=o2_dst[:, q * B : (q + 1) * B], in_=dot[:])
                _kernel_tile(nc, pools, m2_src[q], None,
                             v1buf[:, q * 512 : (q + 1) * 512], store2)

    nc.compile()
    return nc


def _choquet_np(vals: np.ndarray, theta: np.ndarray) -> np.ndarray:
    II, JJ = np.triu_indices(B, k=1)
    n = theta.shape[0]
    xs = vals.reshape(n, B).astype(np.float64)
    t = theta.astype(np.float64)
    e = np.exp(t - t.max(axis=1, keepdims=True))
    m = e / e.sum(axis=1, keepdims=True)
    mins = np.minimum(xs[:, II], xs[:, JJ])
    return (m[:, :B] * xs).sum(axis=1) + (m[:, B:] * mins).sum(axis=1)


def _softmax_f32(theta: np.ndarray) -> np.ndarray:
    t = np.asarray(theta, dtype=np.float32)
    e = np.exp(t - t.max(axis=1, keepdims=True))
    return e / e.sum(axis=1, keepdims=True)


def _fmajor_weights(m: np.ndarray, n_tiles: int) -> np.ndarray:
    """[nodes, 136] f32 softmax -> per-tile feature-major bf16 layout.

    Node n = t*2048 + p*16 + g -> tile t, partition p, col f*16+g with
    features ordered [16 singles ; 120 wrapped-rotation pairs]."""
    import ml_dtypes

    cols = np.concatenate([np.arange(B), B + PAIR_PERM])
    m = m[:, cols]                                   # (nodes, 136) f-order
    m = m.reshape(n_tiles, 128, B, NF)               # (t, p, g, f)
    m = m.transpose(0, 1, 3, 2)                      # (t, p, f, g)
    return np.ascontiguousarray(m.astype(ml_dtypes.bfloat16)).reshape(-1)


_PROG_CACHE: bass.Bass | None = None
LAST_RESULTS = None


def _ensure_ntff_hook() -> None:
    """Provide antenv.axon_hooks + the ctypes NTFF hook when the image
    lacks them, so trace=True produces a perfetto profile under axon."""
    import contextlib
    import ctypes
    import sys
    import types

    try:
        from antenv.axon_hooks import get_axon_ntff_profile_hook  # noqa: F401

        return
    except ImportError:
        pass

    import antenv
    import concourse.bass_utils as bu

    holder = {"h": None}
    mod = types.ModuleType("antenv.axon_hooks")
    mod.set_axon_ntff_profile_hook = lambda h: holder.__setitem__("h", h)
    mod.get_axon_ntff_profile_hook = lambda: holder["h"]
    sys.modules["antenv.axon_hooks"] = mod
    antenv.axon_hooks = mod
    bu.upload_artifacts = lambda tmpdir: ""

    so_path = "/opt/axon/libaxon_pjrt.so"
    try:
        lib = ctypes.CDLL(so_path)
    except OSError:
        return
    if not hasattr(lib, "axon_start_nrt_profile"):
        return
    lib.axon_start_nrt_profile.argtypes = [
        ctypes.POINTER(ctypes.c_int64),
        ctypes.c_size_t,
    ]
    lib.axon_start_nrt_profile.restype = ctypes.c_int64
    lib.axon_stop_nrt_profile.argtypes = [ctypes.c_char_p]
    lib.axon_stop_nrt_profile.restype = ctypes.c_int64

    @contextlib.contextmanager
    def _hook(output_dir, device_ids):
        import jax

        jax.devices()
        if device_ids:
            ids = (ctypes.c_int64 * len(device_ids))(*device_ids)
            rc = lib.axon_start_nrt_profile(ids, len(device_ids))
        else:
            rc = lib.axon_start_nrt_profile(None, 0)
        if rc != 0:
            raise RuntimeError(f"axon_start_nrt_profile rc={rc}")
        try:
            yield
        finally:
            n = lib.axon_stop_nrt_profile(str(output_dir).encode())
            print(f"profile: {n} file(s) written to {output_dir}")

    mod.set_axon_ntff_profile_hook(_hook)


def kernel(x, theta1, theta2, theta3, theta4, theta5, theta6) -> np.ndarray:
    global _PROG_CACHE, LAST_RESULTS
    import ml_dtypes

    x = np.ascontiguousarray(np.asarray(x, dtype=np.float32).reshape(-1))
    m1 = _softmax_f32(np.asarray(theta1, dtype=np.float32))
    m2 = _softmax_f32(np.asarray(theta2, dtype=np.float32))

    if _PROG_CACHE is None:
        _PROG_CACHE = _build_program()
    nc = _PROG_CACHE

    in_maps = []
    for c in range(NCORE):
        xc = x[c * LEAF_PER_CORE : (c + 1) * LEAF_PER_CORE]
        # leaf n = t*32768 + p*256 + g*16 + i -> xs[t, p, i*16+g], dup'd
        xs = xc.reshape(T1, 128, B, B).transpose(0, 1, 3, 2)  # (t,p,i,g)
        xs = xs.reshape(T1, 128, 256).astype(ml_dtypes.bfloat16)
        xd = np.concatenate([xs, xs], axis=2)                 # (t,p,512)

        m1c = _fmajor_weights(m1[c * N1 : (c + 1) * N1], T1)
        # level-2 node j = t*128 + p -> sub-tile q=t//16, col f*16 + (t%16)
        m2c = m2[c * N2 : (c + 1) * N2]                       # (8192, 136)
        m2c = m2c.reshape(64, 128, NF).transpose(1, 0, 2)     # (p, t, f)
        m2c = m2c.reshape(128, Q2, B, NF).transpose(1, 0, 2, 3)  # (q,p,t,f)
        cols = np.concatenate([np.arange(B), B + PAIR_PERM])
        m2c = m2c[:, :, :, cols].transpose(0, 1, 3, 2)        # (q,p,f,t)
        m2c = np.ascontiguousarray(
            m2c.astype(ml_dtypes.bfloat16)).reshape(-1)

        in_maps.append({
            "m1": m1c,
            "xd": np.ascontiguousarray(xd).reshape(-1),
            "m2": m2c,
        })

    trace = os.environ.get("BASS_KERNEL_TRACE", "0") == "1"
    if trace:
        _ensure_ntff_hook()
    res = run_bass_kernel_spmd(nc, in_maps, list(range(NCORE)), trace=trace)
    LAST_RESULTS = res

    # o2[p, t] = level-2 node j = t*128 + p
    l2 = np.concatenate([
        np.asarray(res.results[c]["o2"], dtype=np.float32)
        .reshape(128, 64).T.reshape(-1)
        for c in range(NCORE)
    ])
    vals = l2
    for th in (theta3, theta4, theta5, theta6):
        vals = _choquet_np(vals, np.asarray(th, dtype=np.float32))
    return vals.astype(np.float32).reshape((1,))
